# revision 16
# baseline (speedup 1.0000x reference)
"""Criss-cross attention block (CCNet) Bass/Tile kernel for Trainium2.

Shapes (hardcoded): B=8, C=256, H=W=128, CR=32. Data-parallel over batch:
core b processes image b. Full inputs in, full output out.

Per-core plan (v3):
  P1   : stream x (f32->bf16 cast in DMA), QKV projections with paired
         weight loads. Evac per chunk: ACT [96,512] -> tA (K@0,Q@32,V@64
         h-major), DVE V -> tA[96:128] w-major (per-quarter layout).
         Q replica -> tB[0:32] via one SBUF->SBUF DMA per quarter.
  vts  : V^T stripes via DMA-XBAR transpose (no PE transposes):
         per-quarter [32,4096] -> vts_row[:,q,:,0:32]; after P1 one
         [32,16384] -> vts_col[:,:,0:32]. Ones col at 32 (memset once).
  row  : energies unpacked on T0 (K@0 x Q@0), exp on ACT, apply
         (V^T @ expe) evac via DVE into zin[0:33] (h-major).
  col  : same; apply evac via DVE/ACT CAST into zin[33:66] h-major
         (w-strided dest) so both halves share one rhs AP for P5.
  Z    : Zr=zin[32], Zc=zin[96] both h-major; single DRAM roundtrip per
         Z row to reshape [1,16384]->[128,128]; r = 1/(Zr+Zc); r stored
         to DRAM once, broadcast per h-chunk into rb=tA[0:66].
  P5   : prenorm zin *= rb (chunked, DVE/gpsimd split), ONE 96-deep
         matmul per psum tile (wzT96 = [WzT;0...;WzT] stacked), residual
         add vs x_bf, bf16 staging, cast (bf16->f32) output DMAs.
"""
import sys

sys.path.insert(0, "/opt/trn_rl_repo")

import numpy as np
import ml_dtypes

import concourse.bass as bass
import concourse.mybir as mybir
from concourse import bacc, tile
from concourse.bass_utils import run_bass_kernel_spmd

B, C, H, W, CR = 8, 256, 128, 128, 32
HW = H * W
BF = ml_dtypes.bfloat16

_BUILD_CACHE = {}


def _build(with_qkv_bias: bool, with_z_bias: bool, taps: bool = False):
    nc = bacc.Bacc("TRN2", target_bir_lowering=False, debug=False, num_devices=8)
    dt = mybir.dt
    f32, bf16 = dt.float32, dt.bfloat16

    x_d = nc.dram_tensor("x", [C, HW], f32, kind="ExternalInput").ap()
    wkqvT_d = nc.dram_tensor("wkqvT", [C, 96], bf16, kind="ExternalInput").ap()
    wzT_d = nc.dram_tensor("wzT96", [96, C], bf16, kind="ExternalInput").ap()
    mask_d = nc.dram_tensor("mask8", [128, 8, 128], bf16, kind="ExternalInput").ap()
    if with_qkv_bias:
        bvkq_d = nc.dram_tensor("bvkq", [1, 96], bf16, kind="ExternalInput").ap()
    if with_z_bias:
        bzr_d = nc.dram_tensor("bz_row", [1, C], bf16, kind="ExternalInput").ap()

    zscr_r = nc.dram_tensor("zscr_r", [HW], bf16, kind="Internal").ap()
    zscr_c = nc.dram_tensor("zscr_c", [HW], bf16, kind="Internal").ap()
    rscr = nc.dram_tensor("rscr", [HW], bf16, kind="Internal").ap()
    out_d = nc.dram_tensor("out", [C, HW], f32, kind="ExternalOutput").ap()
    if taps:
        d_tB = nc.dram_tensor("d_tB", [32, HW], bf16, kind="ExternalOutput").ap()
        d_vtsr = nc.dram_tensor("d_vtsr", [128, 4 * 32 * 48], bf16,
                                kind="ExternalOutput").ap()
        d_vtsc = nc.dram_tensor("d_vtsc", [128, 128 * 48], bf16,
                                kind="ExternalOutput").ap()
        d_zin = nc.dram_tensor("d_zin", [97, HW], bf16, kind="ExternalOutput").ap()
        d_r = nc.dram_tensor("d_r", [128, 128], bf16, kind="ExternalOutput").ap()

    with tile.TileContext(nc) as tc:
        with (
            tc.tile_pool(name="persist", bufs=1) as pp,
            tc.tile_pool(name="work", bufs=2) as wp,
            tc.tile_pool(name="outw", bufs=2) as op,
            tc.tile_pool(name="psA", bufs=2, space="PSUM") as pA,
            tc.tile_pool(name="psB", bufs=4, space="PSUM") as pB,
        ):
            # ---- persistent SBUF ----
            x_bf = pp.tile([128, 2, HW], bf16)
            # tA rows: K@0, Q@32, V@64 (h-major), ones@96:112.
            tA = pp.tile([128, H, W], bf16)
            # tB rows: Q@0 replica, V2@64:96 (w-major: free (w, h)), ones@96:112
            tB = pp.tile([112, H, W], bf16)
            # zin rows: row-out 0:32, Zr@32, zero hole 33:64,
            # col-out 64:96 (h-major), Zc@96
            zin = pp.tile([97, H, W], bf16)
            vts_row = pp.tile([128, 4, 32, 48], bf16)  # [w, q, hl, c]; ones@c32
            vts_col = pp.tile([128, 128, 48], bf16)    # [h, w, c]; ones@c32
            wkqvT = pp.tile([128, 2, 96], bf16)
            wzT96 = pp.tile([96, C], bf16)
            mask8 = pp.tile([128, 8, 128], bf16)

            nc.sync.dma_start(out=wkqvT[:], in_=wkqvT_d.rearrange("(a p) m -> p a m", p=128))
            nc.sync.dma_start(out=wzT96[:], in_=wzT_d)
            nc.sync.dma_start(out=mask8[:], in_=mask_d)
            if with_qkv_bias or with_z_bias:
                ones_row = pp.tile([1, 512], bf16)
                nc.vector.memset(ones_row[:], 1.0)
            if with_qkv_bias:
                bvkq = pp.tile([1, 96], bf16)
                nc.sync.dma_start(out=bvkq[:], in_=bvkq_d)
            if with_z_bias:
                bz_row = pp.tile([1, C], bf16)
                nc.sync.dma_start(out=bz_row[:], in_=bzr_d)

            nc.vector.memset(zin[32:64, :, :], 0.0)
            nc.vector.memset(tA[96:112, :, :], 1.0)   # ones rows for row-vts
            nc.vector.memset(tB[96:112, :, :], 1.0)   # ones rows for col-vts

            vrow_src = tA[64:112].rearrange("p a b -> p (a b)")  # [48, HW]
            vcol_src = tB[64:112].rearrange("p a b -> p (a b)")  # [48, HW]

            def attn_batch(b8, row_mode, expe_box):
                """Energies+exp for batch b8 (8 stripes), K@0 x Q@0."""
                s0 = b8 * 8
                ps_e = pA.tile([128, 8, 128], f32, tag="pse")
                ksrc, qsrc = tA[0:32], tB[0:32]
                for j in range(8):
                    s = s0 + j
                    if row_mode:
                        lhsT, rhs = ksrc[:, s, :], qsrc[:, s, :]
                    else:
                        lhsT, rhs = ksrc[:, :, s], qsrc[:, :, s]
                    nc.tensor.matmul(ps_e[:, j, :], lhsT, rhs,
                                     start=True, stop=True)
                expe = wp.tile([128, 8, 128], bf16, tag="expe")
                nc.scalar.activation(expe[:], ps_e[:], mybir.ActivationFunctionType.Exp)
                if not row_mode:
                    nc.vector.tensor_mul(expe[:], expe[:], mask8[:])
                expe_box[b8] = expe

            zc_view = zin[64:97].rearrange("p h w -> p w h")  # [33, W, H]

            def apply_batch(b8, row_mode, expe_box):
                """V^T @ exp for batch b8. Row -> zin[0:33] via gpsimd,
                col -> zin[33:66] h-major (strided) via DVE/ACT."""
                s0 = b8 * 8
                expe = expe_box[b8]
                for half in range(2):
                    ps_a = pB.tile([33, 4, 128], f32, tag="psb", name="ps_a")
                    for jj in range(4):
                        j = half * 4 + jj
                        s = s0 + j
                        if row_mode:
                            lhsT = vts_row[:, s // 32, s % 32, 0:33]
                        else:
                            lhsT = vts_col[:, s, 0:33]
                        nc.tensor.matmul(ps_a[:, jj, :], lhsT,
                                         expe[:, j, :], start=True, stop=True)
                    c0 = s0 + half * 4
                    if row_mode:
                        nc.vector.tensor_copy(zin[0:33, c0:c0 + 4, :], ps_a[:])
                    else:
                        if half == 0:
                            nc.vector.tensor_copy(zc_view[:, c0:c0 + 4, :], ps_a[:])
                        else:
                            nc.scalar.copy(zc_view[:, c0:c0 + 4, :], ps_a[:])
                expe_box[b8] = None

            # ========== P1 + row attention, interleaved by quarters ==========
            expe_box = [None] * 16
            prev_rb = None
            for q in range(4):
                s = q * 4096
                nsub = 4 if q == 0 else 1
                sub = 4096 // nsub
                for si in range(nsub):
                    for half in range(2):
                        s1 = s + si * sub
                        nc.gpsimd.dma_start(
                            out=x_bf[:, half, s1:s1 + sub],
                            in_=x_d[half * 128:(half + 1) * 128, s1:s1 + sub])
                for cp in range(4):  # chunk pairs: shared weight loads
                    ch0 = q * 8 + cp * 2
                    pss = []
                    for ci in range(2):
                        ps = pB.tile([96, 512], f32, tag="psb", name="ps_qkv")
                        pss.append((ps, (ch0 + ci) * 512))
                    for hf in range(2):
                        for ci in range(2):
                            ps, s2 = pss[ci]
                            nc.tensor.matmul(
                                ps[:], wkqvT[:, hf, :], x_bf[:, hf, s2:s2 + 512],
                                start=(hf == 0),
                                stop=(hf == 1) and not with_qkv_bias)
                    if with_qkv_bias:
                        for ci in range(2):
                            ps, s2 = pss[ci]
                            nc.tensor.matmul(ps[:], bvkq[:], ones_row[:],
                                             start=False, stop=True)
                    for ci in range(2):
                        ch = ch0 + ci
                        ps, s2 = pss[ci]
                        h0 = ch * 4
                        ps3 = ps[:].rearrange("p (a b) -> p a b", b=128)
                        nc.scalar.copy(tA[0:96, h0:h0 + 4, :], ps3[0:96])
                        v2dst = tB[64:96, :, h0:h0 + 4].rearrange("p w h -> p h w")
                        nc.vector.tensor_copy(v2dst, ps3[64:96])
                # Q replica for this quarter via one SBUF->SBUF DMA
                qh0 = q * 32
                nc.sync.dma_start(out=tB[0:32, qh0:qh0 + 32, :],
                                  in_=tA[32:64, qh0:qh0 + 32, :])
                # V^T stripes for this quarter via DMA XBAR transpose
                nc.sync.dma_start(out=vts_row[:, q],
                                  in_=vrow_src[:, s:s + 4096], transpose=True)
                # row attention for this quarter (software-pipelined)
                for bl in range(4):
                    b8 = q * 4 + bl
                    attn_batch(b8, True, expe_box)
                    if prev_rb is not None:
                        apply_batch(prev_rb, True, expe_box)
                    prev_rb = b8
            apply_batch(prev_rb, True, expe_box)
            # Zr reshape roundtrip can run during the col phase
            nc.sync.dma_start(out=zscr_r.rearrange("(p f) -> p f", p=1),
                              in_=zin[32:33, :, :].rearrange("p a b -> p (a b)"))
            zr2 = wp.tile([128, 128], bf16, tag="zr2", bufs=1)
            nc.sync.dma_start(out=zr2[:], in_=zscr_r.rearrange("(p f) -> p f", p=128))
            # col-mode V^T via one big DMA XBAR transpose
            nc.sync.dma_start(out=vts_col[:], in_=vcol_src, transpose=True)

            # ========== column attention ==========
            prev = None
            for wb in range(17):
                if wb < 16:
                    attn_batch(wb, False, expe_box)
                if prev is not None:
                    apply_batch(prev, False, expe_box)
                prev = wb if wb < 16 else None

            # ========== Z -> r (both Z rows are h-major now) ==========
            nc.sync.dma_start(out=zscr_c.rearrange("(p f) -> p f", p=1),
                              in_=zin[96:97, :, :].rearrange("p a b -> p (a b)"))
            zc2 = wp.tile([128, 128], bf16, tag="zc2", bufs=1)
            nc.sync.dma_start(out=zc2[:], in_=zscr_c.rearrange("(p f) -> p f", p=128))
            zs = wp.tile([128, 128], f32, tag="zs", bufs=1)
            nc.vector.tensor_add(zs[:], zr2[:], zc2[:])
            rsq = wp.tile([128, 128], f32, tag="rsq", bufs=1)
            nc.vector.reciprocal(rsq[:], zs[:])
            r_bf = wp.tile([128, 128], bf16, tag="r_bf", bufs=1)
            nc.vector.tensor_copy(r_bf[:], rsq[:])
            nc.sync.dma_start(out=rscr.rearrange("(p f) -> p f", p=128), in_=r_bf[:])

            # ========== P5: prenorm, 66-deep Wz, residual, store ==========
            rb = tA[0:97, :, :]              # r broadcast target (dead K/Q/V)
            src_r = rscr.rearrange("(a b) -> a b", b=128)
            chunks = [(hc * 16, 16) for hc in range(7)] + [(112, 8), (120, 8)]
            for ck, (h0, hn) in enumerate(chunks):
                # broadcast r rows h0:h0+hn to partitions 0-65, then prenorm
                sl = src_r[h0:h0 + hn, :]
                bc = bass.AP(tensor=sl.tensor, offset=sl.offset,
                             ap=[[0, 97]] + list(sl.ap))
                nc.gpsimd.dma_start(out=rb[:, h0:h0 + hn, :], in_=bc)
                if ck % 2 == 0:
                    nc.vector.tensor_mul(zin[:, h0:h0 + hn, :],
                                         zin[:, h0:h0 + hn, :],
                                         rb[:, h0:h0 + hn, :])
                else:
                    nc.gpsimd.tensor_mul(zin[:, h0:h0 + hn, :],
                                         zin[:, h0:h0 + hn, :],
                                         rb[:, h0:h0 + hn, :])
                ofs = []
                for half in range(2):
                    of = op.tile([128, 16, 128], bf16, tag="of", name="of")
                    ofs.append(of)
                for wt in range(4):          # w-tiles of 32 cols
                    w0 = wt * 32
                    rhs = zin[0:96, h0:h0 + hn, w0:w0 + 32]
                    for half in range(2):
                        ps_f = pB.tile([128, hn * 32], f32, tag="psb", name="ps_f")
                        wzh = wzT96[:, half * 128:(half + 1) * 128]
                        nc.tensor.matmul(ps_f[:], wzh, rhs,
                                         start=True, stop=not with_z_bias)
                        if with_z_bias:
                            nc.tensor.matmul(
                                ps_f[:], bz_row[:, half * 128:(half + 1) * 128],
                                ones_row[:, 0:hn * 32], start=False, stop=True)
                        x_t = x_bf[:, half, :].rearrange(
                            "p (a b) -> p a b", b=128)[:, h0:h0 + hn, w0:w0 + 32]
                        dst = ofs[half][:, 0:hn, w0:w0 + 32]
                        psv = ps_f[:].rearrange("p (a b) -> p a b", b=32)
                        if wt % 2 == 0:
                            nc.vector.tensor_add(dst, psv, x_t)
                        else:
                            nc.scalar.copy(dst, psv)
                            nc.gpsimd.tensor_add(dst, dst, x_t)
                for half in range(2):
                    nc.gpsimd.dma_start(
                        out=out_d[half * 128:(half + 1) * 128,
                                  h0 * 128:(h0 + hn) * 128],
                        in_=ofs[half][:, 0:hn, :].rearrange("p a b -> p (a b)"))
            if taps:
                nc.sync.dma_start(out=d_tB, in_=tB[0:32].rearrange("p a b -> p (a b)"))
                nc.sync.dma_start(out=d_vtsr,
                                  in_=vts_row[:].rearrange("p a b c -> p (a b c)"))
                nc.sync.dma_start(out=d_vtsc,
                                  in_=vts_col[:].rearrange("p a b -> p (a b)"))
                nc.sync.dma_start(out=d_zin,
                                  in_=zin[:].rearrange("p a b -> p (a b)"))
                nc.sync.dma_start(out=d_r, in_=r_bf[:])
    nc.compile()
    return nc


def _host_prep(Wq, bq, Wk, bk, Wv, bv, Wz, bz):
    wkqvT = np.ascontiguousarray(
        np.concatenate([Wk, Wq, Wv], axis=0).T).astype(BF)          # (256, 96)
    wzT = np.ascontiguousarray(Wz.T).astype(np.float32)              # (32, 256)
    wzT96 = np.zeros((96, C), np.float32)
    wzT96[0:32] = wzT
    wzT96[64:96] = wzT
    wzT96 = wzT96.astype(BF)
    bz_row = np.asarray(bz, np.float32).reshape(1, C).astype(BF)
    eye = np.eye(128, dtype=np.float32)
    mask8 = np.ascontiguousarray(
        np.broadcast_to((1.0 - eye)[:, None, :], (128, 8, 128))).astype(BF)
    bvkq = np.concatenate([bk, bq, bv]).reshape(1, 96).astype(BF)
    return wkqvT, wzT96, bz_row, mask8, bvkq


def kernel(x, Wq, bq, Wk, bk, Wv, bv, Wz, bz):
    x = np.asarray(x, np.float32)
    wkqvT, wzT96, bz_row, mask8, bvkq = _host_prep(
        np.asarray(Wq, np.float32), np.asarray(bq, np.float32),
        np.asarray(Wk, np.float32), np.asarray(bk, np.float32),
        np.asarray(Wv, np.float32), np.asarray(bv, np.float32),
        np.asarray(Wz, np.float32), np.asarray(bz, np.float32))
    with_qkv_bias = bool(np.any(bvkq.astype(np.float32) != 0.0))
    with_z_bias = bool(np.any(bz_row.astype(np.float32) != 0.0))

    key = (with_qkv_bias, with_z_bias)
    if key not in _BUILD_CACHE:
        _BUILD_CACHE[key] = _build(*key)
    nc = _BUILD_CACHE[key]

    in_maps = []
    for b in range(B):
        m = dict(
            x=np.ascontiguousarray(x[b].reshape(C, HW)),
            wkqvT=wkqvT, wzT96=wzT96, mask8=mask8,
        )
        if with_qkv_bias:
            m["bvkq"] = bvkq
        if with_z_bias:
            m["bz_row"] = bz_row
        in_maps.append(m)

    res = run_bass_kernel_spmd(nc, in_maps, core_ids=list(range(8)))
    out = np.stack([res.results[b]["out"].reshape(C, H, W) for b in range(B)])
    return out


# revision 18
# speedup vs baseline: 1.0178x; 1.0178x over previous
"""Criss-cross attention block (CCNet) Bass/Tile kernel for Trainium2.

Shapes (hardcoded): B=8, C=256, H=W=128, CR=32. Data-parallel over batch:
core b processes image b. Full inputs in, full output out.

Per-core plan (v3):
  P1   : stream x (f32->bf16 cast in DMA), QKV projections with paired
         weight loads. Evac per chunk: ACT [96,512] -> tA (K@0,Q@32,V@64
         h-major), DVE V -> tA[96:128] w-major (per-quarter layout).
         Q replica -> tB[0:32] via one SBUF->SBUF DMA per quarter.
  vts  : V^T stripes via DMA-XBAR transpose (no PE transposes):
         per-quarter [32,4096] -> vts_row[:,q,:,0:32]; after P1 one
         [32,16384] -> vts_col[:,:,0:32]. Ones col at 32 (memset once).
  row  : energies unpacked on T0 (K@0 x Q@0), exp on ACT, apply
         (V^T @ expe) evac via DVE into zin[0:33] (h-major).
  col  : same; apply evac via DVE/ACT CAST into zin[33:66] h-major
         (w-strided dest) so both halves share one rhs AP for P5.
  Z    : Zr=zin[32], Zc=zin[96] both h-major; single DRAM roundtrip per
         Z row to reshape [1,16384]->[128,128]; r = 1/(Zr+Zc); r stored
         to DRAM once, broadcast per h-chunk into rb=tA[0:66].
  P5   : prenorm zin *= rb (chunked, DVE/gpsimd split), ONE 96-deep
         matmul per psum tile (wzT96 = [WzT;0...;WzT] stacked), residual
         add vs x_bf, bf16 staging, cast (bf16->f32) output DMAs.
"""
import sys

sys.path.insert(0, "/opt/trn_rl_repo")

import numpy as np
import ml_dtypes

import concourse.bass as bass
import concourse.mybir as mybir
from concourse import bacc, tile
from concourse.bass_utils import run_bass_kernel_spmd

B, C, H, W, CR = 8, 256, 128, 128, 32
HW = H * W
BF = ml_dtypes.bfloat16

_BUILD_CACHE = {}


def _build(with_qkv_bias: bool, with_z_bias: bool, taps: bool = False):
    nc = bacc.Bacc("TRN2", target_bir_lowering=False, debug=False, num_devices=8)
    dt = mybir.dt
    f32, bf16 = dt.float32, dt.bfloat16

    x_d = nc.dram_tensor("x", [C, HW], f32, kind="ExternalInput").ap()
    wkqvT_d = nc.dram_tensor("wkqvT", [C, 96], bf16, kind="ExternalInput").ap()
    wzT_d = nc.dram_tensor("wzT96", [96, C], bf16, kind="ExternalInput").ap()
    mask_d = nc.dram_tensor("mask8", [128, 8, 128], bf16, kind="ExternalInput").ap()
    czo_d = nc.dram_tensor("czo", [2, 512], bf16, kind="ExternalInput").ap()
    if with_qkv_bias:
        bvkq_d = nc.dram_tensor("bvkq", [1, 96], bf16, kind="ExternalInput").ap()
    if with_z_bias:
        bzr_d = nc.dram_tensor("bz_row", [1, C], bf16, kind="ExternalInput").ap()

    zscr_r = nc.dram_tensor("zscr_r", [HW], bf16, kind="Internal").ap()
    zscr_c = nc.dram_tensor("zscr_c", [HW], bf16, kind="Internal").ap()
    rscr = nc.dram_tensor("rscr", [HW], bf16, kind="Internal").ap()
    out_d = nc.dram_tensor("out", [C, HW], f32, kind="ExternalOutput").ap()
    if taps:
        d_tB = nc.dram_tensor("d_tB", [32, HW], bf16, kind="ExternalOutput").ap()
        d_vtsr = nc.dram_tensor("d_vtsr", [128, 4 * 32 * 48], bf16,
                                kind="ExternalOutput").ap()
        d_vtsc = nc.dram_tensor("d_vtsc", [128, 128 * 48], bf16,
                                kind="ExternalOutput").ap()
        d_zin = nc.dram_tensor("d_zin", [97, HW], bf16, kind="ExternalOutput").ap()
        d_r = nc.dram_tensor("d_r", [128, 128], bf16, kind="ExternalOutput").ap()

    with tile.TileContext(nc) as tc:
        with (
            tc.tile_pool(name="persist", bufs=1) as pp,
            tc.tile_pool(name="work", bufs=2) as wp,
            tc.tile_pool(name="outw", bufs=2) as op,
            tc.tile_pool(name="psA", bufs=2, space="PSUM") as pA,
            tc.tile_pool(name="psB", bufs=4, space="PSUM") as pB,
        ):
            # ---- persistent SBUF ----
            x_bf = pp.tile([128, 2, HW], bf16)
            # tA rows: K@0, Q@32, V@64 (h-major), ones@96:112.
            tA = pp.tile([128, H, W], bf16)
            # tB rows: Q@0 replica, V2@64:96 (w-major: free (w, h)), ones@96:112
            tB = pp.tile([112, H, W], bf16)
            # zin rows: row-out 0:32, Zr@32, zero hole 33:64,
            # col-out 64:96 (h-major), Zc@96
            zin = pp.tile([97, H, W], bf16)
            vts_row = pp.tile([128, 4, 32, 48], bf16)  # [w, q, hl, c]; ones@c32
            vts_col = pp.tile([128, 128, 48], bf16)    # [h, w, c]; ones@c32
            wkqvT = pp.tile([128, 2, 96], bf16)
            wzT96 = pp.tile([96, C], bf16)
            mask8 = pp.tile([128, 8, 128], bf16)

            nc.sync.dma_start(out=wkqvT[:], in_=wkqvT_d.rearrange("(a p) m -> p a m", p=128))
            nc.sync.dma_start(out=wzT96[:], in_=wzT_d)
            nc.sync.dma_start(out=mask8[:], in_=mask_d)
            if with_qkv_bias or with_z_bias:
                ones_row = pp.tile([1, 512], bf16)
                nc.vector.memset(ones_row[:], 1.0)
            if with_qkv_bias:
                bvkq = pp.tile([1, 96], bf16)
                nc.sync.dma_start(out=bvkq[:], in_=bvkq_d)
            if with_z_bias:
                bz_row = pp.tile([1, C], bf16)
                nc.sync.dma_start(out=bz_row[:], in_=bzr_d)

            # consts via DMA broadcast (DVE memset of [*,16384] costs ~14us)
            zrow = czo_d[0:1, :]
            orow = czo_d[1:2, :]
            def _bcast(dst, row, np_, nf):
                bcap = bass.AP(tensor=row.tensor, offset=row.offset,
                               ap=[[0, np_], [0, nf // 512], [1, 512]])
                nc.sync.dma_start(out=dst, in_=bcap)
            _bcast(zin[32:64, :, :], zrow, 32, HW)
            _bcast(tA[96:112, :, :], orow, 16, HW)
            _bcast(tB[96:112, :, :], orow, 16, HW)

            vrow_src = tA[64:112].rearrange("p a b -> p (a b)")  # [48, HW]
            vcol_src = tB[64:112].rearrange("p a b -> p (a b)")  # [48, HW]

            def attn_batch(b8, row_mode, expe_box):
                """Energies+exp for batch b8 (8 stripes), K@0 x Q@0."""
                s0 = b8 * 8
                ps_e = pA.tile([128, 8, 128], f32, tag="pse")
                ksrc, qsrc = tA[0:32], tB[0:32]
                for j in range(8):
                    s = s0 + j
                    if row_mode:
                        lhsT, rhs = ksrc[:, s, :], qsrc[:, s, :]
                    else:
                        lhsT, rhs = ksrc[:, :, s], qsrc[:, :, s]
                    nc.tensor.matmul(ps_e[:, j, :], lhsT, rhs,
                                     start=True, stop=True)
                expe = wp.tile([128, 8, 128], bf16, tag="expe")
                nc.scalar.activation(expe[:], ps_e[:], mybir.ActivationFunctionType.Exp)
                if not row_mode:
                    nc.vector.tensor_mul(expe[:], expe[:], mask8[:])
                expe_box[b8] = expe

            zc_view = zin[64:97].rearrange("p h w -> p w h")  # [33, W, H]

            def apply_batch(b8, row_mode, expe_box):
                """V^T @ exp for batch b8. Row -> zin[0:33] via gpsimd,
                col -> zin[33:66] h-major (strided) via DVE/ACT."""
                s0 = b8 * 8
                expe = expe_box[b8]
                for half in range(2):
                    ps_a = pB.tile([33, 4, 128], f32, tag="psb", name="ps_a")
                    for jj in range(4):
                        j = half * 4 + jj
                        s = s0 + j
                        if row_mode:
                            lhsT = vts_row[:, s // 32, s % 32, 0:33]
                        else:
                            lhsT = vts_col[:, s, 0:33]
                        nc.tensor.matmul(ps_a[:, jj, :], lhsT,
                                         expe[:, j, :], start=True, stop=True)
                    c0 = s0 + half * 4
                    if row_mode:
                        nc.vector.tensor_copy(zin[0:33, c0:c0 + 4, :], ps_a[:])
                    else:
                        zcs = wp.tile([33, 4, 128], bf16, tag="zcs", bufs=3,
                                      name="zcs")
                        if half == 0:
                            nc.vector.tensor_copy(zcs[:], ps_a[:])
                        else:
                            nc.scalar.copy(zcs[:], ps_a[:])
                        nc.gpsimd.tensor_copy(zc_view[:, c0:c0 + 4, :], zcs[:])
                expe_box[b8] = None

            # ========== P1 + row attention, interleaved by quarters ==========
            expe_box = [None] * 16
            prev_rb = None
            for q in range(4):
                s = q * 4096
                nsub = 4 if q == 0 else 1
                sub = 4096 // nsub
                for si in range(nsub):
                    for half in range(2):
                        s1 = s + si * sub
                        nc.gpsimd.dma_start(
                            out=x_bf[:, half, s1:s1 + sub],
                            in_=x_d[half * 128:(half + 1) * 128, s1:s1 + sub])
                for cp in range(4):  # chunk pairs: shared weight loads
                    ch0 = q * 8 + cp * 2
                    pss = []
                    for ci in range(2):
                        ps = pB.tile([96, 512], f32, tag="psb", name="ps_qkv")
                        pss.append((ps, (ch0 + ci) * 512))
                    for hf in range(2):
                        for ci in range(2):
                            ps, s2 = pss[ci]
                            nc.tensor.matmul(
                                ps[:], wkqvT[:, hf, :], x_bf[:, hf, s2:s2 + 512],
                                start=(hf == 0),
                                stop=(hf == 1) and not with_qkv_bias)
                    if with_qkv_bias:
                        for ci in range(2):
                            ps, s2 = pss[ci]
                            nc.tensor.matmul(ps[:], bvkq[:], ones_row[:],
                                             start=False, stop=True)
                    for ci in range(2):
                        ch = ch0 + ci
                        ps, s2 = pss[ci]
                        h0 = ch * 4
                        ps3 = ps[:].rearrange("p (a b) -> p a b", b=128)
                        nc.scalar.copy(tA[0:96, h0:h0 + 4, :], ps3[0:96])
                # Q replica + w-major V2 for this quarter via SBUF->SBUF DMAs
                qh0 = q * 32
                nc.sync.dma_start(out=tB[0:32, qh0:qh0 + 32, :],
                                  in_=tA[32:64, qh0:qh0 + 32, :])
                nc.gpsimd.tensor_copy(
                    tB[64:96, :, qh0:qh0 + 32],
                    tA[64:96, qh0:qh0 + 32, :].rearrange("p h w -> p w h"))
                # V^T stripes for this quarter via DMA XBAR transpose
                nc.sync.dma_start(out=vts_row[:, q],
                                  in_=vrow_src[:, s:s + 4096], transpose=True)
                # row attention for this quarter (software-pipelined)
                for bl in range(4):
                    b8 = q * 4 + bl
                    attn_batch(b8, True, expe_box)
                    if prev_rb is not None:
                        apply_batch(prev_rb, True, expe_box)
                    prev_rb = b8
            apply_batch(prev_rb, True, expe_box)
            # Zr reshape roundtrip can run during the col phase
            nc.sync.dma_start(out=zscr_r.rearrange("(p f) -> p f", p=1),
                              in_=zin[32:33, :, :].rearrange("p a b -> p (a b)"))
            zr2 = wp.tile([128, 128], bf16, tag="zr2", bufs=1)
            nc.sync.dma_start(out=zr2[:], in_=zscr_r.rearrange("(p f) -> p f", p=128))
            # col-mode V^T via one big DMA XBAR transpose
            nc.sync.dma_start(out=vts_col[:], in_=vcol_src, transpose=True)

            # ========== column attention ==========
            prev = None
            for wb in range(17):
                if wb < 16:
                    attn_batch(wb, False, expe_box)
                if prev is not None:
                    apply_batch(prev, False, expe_box)
                prev = wb if wb < 16 else None

            # ========== Z -> r (both Z rows are h-major now) ==========
            nc.sync.dma_start(out=zscr_c.rearrange("(p f) -> p f", p=1),
                              in_=zin[96:97, :, :].rearrange("p a b -> p (a b)"))
            zc2 = wp.tile([128, 128], bf16, tag="zc2", bufs=1)
            nc.sync.dma_start(out=zc2[:], in_=zscr_c.rearrange("(p f) -> p f", p=128))
            zs = wp.tile([128, 128], f32, tag="zs", bufs=1)
            nc.vector.tensor_add(zs[:], zr2[:], zc2[:])
            rsq = wp.tile([128, 128], f32, tag="rsq", bufs=1)
            nc.vector.reciprocal(rsq[:], zs[:])
            r_bf = wp.tile([128, 128], bf16, tag="r_bf", bufs=1)
            nc.vector.tensor_copy(r_bf[:], rsq[:])
            nc.sync.dma_start(out=rscr.rearrange("(p f) -> p f", p=128), in_=r_bf[:])

            # ========== P5: prenorm, 66-deep Wz, residual, store ==========
            rb = tA[0:97, :, :]              # r broadcast target (dead K/Q/V)
            src_r = rscr.rearrange("(a b) -> a b", b=128)
            chunks = [(hc * 16, 16) for hc in range(7)] + [(112, 8), (120, 8)]
            for ck, (h0, hn) in enumerate(chunks):
                # broadcast r rows h0:h0+hn to partitions 0-65, then prenorm
                sl = src_r[h0:h0 + hn, :]
                bc = bass.AP(tensor=sl.tensor, offset=sl.offset,
                             ap=[[0, 97]] + list(sl.ap))
                nc.gpsimd.dma_start(out=rb[:, h0:h0 + hn, :], in_=bc)
                if ck % 2 == 0:
                    nc.vector.tensor_mul(zin[:, h0:h0 + hn, :],
                                         zin[:, h0:h0 + hn, :],
                                         rb[:, h0:h0 + hn, :])
                else:
                    nc.gpsimd.tensor_mul(zin[:, h0:h0 + hn, :],
                                         zin[:, h0:h0 + hn, :],
                                         rb[:, h0:h0 + hn, :])
                ofs = []
                for half in range(2):
                    of = op.tile([128, 16, 128], bf16, tag="of", name="of")
                    ofs.append(of)
                for wt in range(4):          # w-tiles of 32 cols
                    w0 = wt * 32
                    rhs = zin[0:96, h0:h0 + hn, w0:w0 + 32]
                    for half in range(2):
                        ps_f = pB.tile([128, hn * 32], f32, tag="psb", name="ps_f")
                        wzh = wzT96[:, half * 128:(half + 1) * 128]
                        nc.tensor.matmul(ps_f[:], wzh, rhs,
                                         start=True, stop=not with_z_bias)
                        if with_z_bias:
                            nc.tensor.matmul(
                                ps_f[:], bz_row[:, half * 128:(half + 1) * 128],
                                ones_row[:, 0:hn * 32], start=False, stop=True)
                        x_t = x_bf[:, half, :].rearrange(
                            "p (a b) -> p a b", b=128)[:, h0:h0 + hn, w0:w0 + 32]
                        dst = ofs[half][:, 0:hn, w0:w0 + 32]
                        psv = ps_f[:].rearrange("p (a b) -> p a b", b=32)
                        if wt % 2 == 0:
                            nc.vector.tensor_add(dst, psv, x_t)
                        else:
                            nc.scalar.copy(dst, psv)
                            nc.gpsimd.tensor_add(dst, dst, x_t)
                for half in range(2):
                    nc.gpsimd.dma_start(
                        out=out_d[half * 128:(half + 1) * 128,
                                  h0 * 128:(h0 + hn) * 128],
                        in_=ofs[half][:, 0:hn, :].rearrange("p a b -> p (a b)"))
            if taps:
                nc.sync.dma_start(out=d_tB, in_=tB[0:32].rearrange("p a b -> p (a b)"))
                nc.sync.dma_start(out=d_vtsr,
                                  in_=vts_row[:].rearrange("p a b c -> p (a b c)"))
                nc.sync.dma_start(out=d_vtsc,
                                  in_=vts_col[:].rearrange("p a b -> p (a b)"))
                nc.sync.dma_start(out=d_zin,
                                  in_=zin[:].rearrange("p a b -> p (a b)"))
                nc.sync.dma_start(out=d_r, in_=r_bf[:])
    nc.compile()
    return nc


def _host_prep(Wq, bq, Wk, bk, Wv, bv, Wz, bz):
    wkqvT = np.ascontiguousarray(
        np.concatenate([Wk, Wq, Wv], axis=0).T).astype(BF)          # (256, 96)
    wzT = np.ascontiguousarray(Wz.T).astype(np.float32)              # (32, 256)
    wzT96 = np.zeros((96, C), np.float32)
    wzT96[0:32] = wzT
    wzT96[64:96] = wzT
    wzT96 = wzT96.astype(BF)
    bz_row = np.asarray(bz, np.float32).reshape(1, C).astype(BF)
    eye = np.eye(128, dtype=np.float32)
    mask8 = np.ascontiguousarray(
        np.broadcast_to((1.0 - eye)[:, None, :], (128, 8, 128))).astype(BF)
    bvkq = np.concatenate([bk, bq, bv]).reshape(1, 96).astype(BF)
    czo = np.zeros((2, 512), np.float32)
    czo[1] = 1.0
    czo = czo.astype(BF)
    return wkqvT, wzT96, bz_row, mask8, bvkq, czo


def kernel(x, Wq, bq, Wk, bk, Wv, bv, Wz, bz):
    x = np.asarray(x, np.float32)
    wkqvT, wzT96, bz_row, mask8, bvkq, czo = _host_prep(
        np.asarray(Wq, np.float32), np.asarray(bq, np.float32),
        np.asarray(Wk, np.float32), np.asarray(bk, np.float32),
        np.asarray(Wv, np.float32), np.asarray(bv, np.float32),
        np.asarray(Wz, np.float32), np.asarray(bz, np.float32))
    with_qkv_bias = bool(np.any(bvkq.astype(np.float32) != 0.0))
    with_z_bias = bool(np.any(bz_row.astype(np.float32) != 0.0))

    key = (with_qkv_bias, with_z_bias)
    if key not in _BUILD_CACHE:
        _BUILD_CACHE[key] = _build(*key)
    nc = _BUILD_CACHE[key]

    in_maps = []
    for b in range(B):
        m = dict(
            x=np.ascontiguousarray(x[b].reshape(C, HW)),
            wkqvT=wkqvT, wzT96=wzT96, mask8=mask8, czo=czo,
        )
        if with_qkv_bias:
            m["bvkq"] = bvkq
        if with_z_bias:
            m["bz_row"] = bz_row
        in_maps.append(m)

    res = run_bass_kernel_spmd(nc, in_maps, core_ids=list(range(8)))
    out = np.stack([res.results[b]["out"].reshape(C, H, W) for b in range(B)])
    return out
